# revision 1
# baseline (speedup 1.0000x reference)
"""Multi-head attention Trainium2 kernel (8 NeuronCores, tensor-parallel over heads).

Strategy:
  - 16 heads / 8 cores = 2 heads per core. x is replicated; Wq/Wk/Wv sharded by
    head; Wp row-sharded (contraction dim). Each core computes a partial
    projection output [B*T, D]; the host sums the 8 partials (+bias).
  - On chip, all contractions need the contracted dim on SBUF partitions, so the
    host passes xT = x.reshape(BT, D).T and per-core transposed weight slices.
  - qT/kT are computed packed [128 = 2 heads x 64, BT]. Scores are computed
    transposed (s on partitions, t on free) so softmax normalization can ride
    the attn@v matmul: lhsT = [v | ones] gives out rows 0..63 = unnormalized
    out^T and row 64 = the softmax denominator Z. Softmax is computed without
    max subtraction (scores are O(10), exp stays in fp32 range).
  - Causality: only lower-triangular [128s x 512t] blocks are computed; the 4
    blocks per t-block straddling the diagonal share one [128,128] staircase
    mask (applied multiplicatively after exp) plus a memset of fully-masked
    columns.
  - Per-stage precision: SBUF tiles are all f32; stages listed in the config
    bitcast their matmul operands to float32r (TF32-like, 4x faster at N>=256).
"""

import numpy as np

B, T, D, H, HD = 2, 2048, 1024, 16, 64
NCORES = 8
HPC = H // NCORES          # heads per core = 2
CH = HPC * HD              # channels per core = 128
BT = B * T

_CACHE = {}

# which matmul stages run in float32r per named config
_CFGS = {
    "f32":  frozenset(),
    "mix":  frozenset({"v", "tr", "av", "bcast", "proj"}),
    "f32r": frozenset({"qk", "v", "tr", "scores", "av", "bcast", "proj"}),
}


def _build(b, t, d, rset):
    """Build + compile the per-core Bass program."""
    import os
    import concourse.tile as tile
    from concourse import bacc, mybir
    from concourse.masks import make_identity
    from contextlib import ExitStack

    phases = os.environ.get("KERNEL_PHASES", "123")

    f32 = mybir.dt.float32
    f32r = mybir.dt.float32r

    def mm(out, lhsT, rhs, stage, **kw):
        if stage in rset:
            lhsT = lhsT.bitcast(f32r)
            rhs = rhs.bitcast(f32r)
        nc.tensor.matmul(out, lhsT, rhs, **kw)

    bt = b * t
    KT = d // 128            # k-tiles over the model dim
    TBLK = min(512, t)       # t-block width for scores/attn
    NJ = t // TBLK           # t-blocks per batch
    NSB = bt // 128          # 128-row s-blocks over B*T
    SPT = TBLK // 128        # s-blocks per t-block

    nc = bacc.Bacc("TRN2", target_bir_lowering=False, debug=False)

    xT = nc.dram_tensor("xT", [d, bt], f32, kind="ExternalInput").ap()
    wq = nc.dram_tensor("wq", [d, CH], f32, kind="ExternalInput").ap()
    wk = nc.dram_tensor("wk", [d, CH], f32, kind="ExternalInput").ap()
    wv = nc.dram_tensor("wv", [d, CH], f32, kind="ExternalInput").ap()
    wp = nc.dram_tensor("wp", [CH, d], f32, kind="ExternalInput").ap()
    out_p = nc.dram_tensor("out_p", [bt, d], f32, kind="ExternalOutput").ap()

    with tile.TileContext(nc) as tc, ExitStack() as top:
        persist = top.enter_context(tc.tile_pool(name="persist", bufs=1))

        # ---- persistent tiles ----
        qT_sb = persist.tile([128, bt], f32, tag="qT")
        kT_sb = persist.tile([128, bt], f32, tag="kT")
        # [v_h0 | 1 | pad | v_h1 | 1 | pad] per 128-row s-block
        vaug = persist.tile([128, NSB, 66 * HPC], f32, tag="vaug")
        outT_sb = persist.tile([128, bt], f32, tag="outT")
        wq_sb = persist.tile([128, KT, CH], f32, tag="wq")
        wk_sb = persist.tile([128, KT, CH], f32, tag="wk")
        wv_sb = persist.tile([128, KT, CH], f32, tag="wv")
        wp_sb = persist.tile([128, d], f32, tag="wp")
        ident = persist.tile([128, 128], f32, tag="ident")
        mask = persist.tile([128, 128], f32, tag="mask")
        ones1 = persist.tile([65, HD], f32, tag="ones1")

        make_identity(nc, ident[:])
        nc.gpsimd.memset(vaug[:], 1.0)
        nc.gpsimd.memset(ones1[:], 1.0)
        # staircase mask: keep (p <= c), i.e. upper-triangular incl. diagonal
        nc.gpsimd.memset(mask[:], 1.0)
        nc.gpsimd.affine_select(
            out=mask[:], in_=mask[:],
            compare_op=mybir.AluOpType.is_ge,
            fill=0.0, base=0,
            # iota = -p + c ; keep when >= 0
            pattern=[[1, 128]], channel_multiplier=-1,
        )

        for w_ap, w_sb in ((wq, wq_sb), (wk, wk_sb), (wv, wv_sb)):
            nc.gpsimd.dma_start(
                out=w_sb[:],
                in_=w_ap.rearrange("(kt p) m -> p kt m", p=128),
            )
        nc.gpsimd.dma_start(out=wp_sb[:], in_=wp)

        # ---- merged loop: per (batch, t-block): QKV -> attention -> proj ----
        # Attention for block j of batch bb needs q columns of block j and
        # k/v columns of blocks 0..j (same batch) -- all computed by the time
        # block j's QKV is done, so one fused loop pipelines everything:
        # xT loads prefetch under attention PE work, and output stores drain
        # under the next block's compute.
        PW = min(512, d)
        NIB = d // PW
        with ExitStack() as body:
            xpool = body.enter_context(tc.tile_pool(name="xpool", bufs=3))
            vtpool = body.enter_context(tc.tile_pool(name="vtpool", bufs=2))
            npool = body.enter_context(tc.tile_pool(name="npool", bufs=8))
            zpool = body.enter_context(tc.tile_pool(name="zpool", bufs=2))
            tmpool = body.enter_context(tc.tile_pool(name="tmpool", bufs=2))
            opool = body.enter_context(tc.tile_pool(name="opool", bufs=4))
            # PSUM budget (8 banks): qkv 2 + scores/bcast 2 + av 2 + tr/proj 2
            ps_qkv = body.enter_context(tc.tile_pool(name="ps_qkv", bufs=2, space="PSUM"))
            ps_s = body.enter_context(tc.tile_pool(name="ps_s", bufs=2, space="PSUM"))
            ps_av = body.enter_context(tc.tile_pool(name="ps_av", bufs=2, space="PSUM"))
            ps_tp = body.enter_context(tc.tile_pool(name="ps_tp", bufs=2, space="PSUM"))

            def emit_qkv(bb, j):
                col0 = bb * t + j * TBLK
                tsl = slice(col0, col0 + TBLK)
                xt = xpool.tile([128, KT, TBLK], f32, tag="xt", name=f"xt_{bb}_{j}")
                for kt in range(KT):
                    nc.sync.dma_start(
                        out=xt[:, kt, :],
                        in_=xT[kt * 128:(kt + 1) * 128, tsl],
                    )
                for w_sb, dst, stg in ((wq_sb, qT_sb, "qk"), (wk_sb, kT_sb, "qk")):
                    ps = ps_qkv.tile([128, TBLK], f32, tag="ps_qkv",
                                     name=f"psq_{bb}_{j}_{stg}_{dst.name}")
                    for kt in range(KT):
                        mm(ps[:], w_sb[:, kt, :], xt[:, kt, :], stg,
                           start=(kt == 0), stop=(kt == KT - 1))
                    nc.vector.tensor_copy(dst[:, tsl], ps[:])
                ps = ps_qkv.tile([128, TBLK], f32, tag="ps_qkv", name=f"psv_{bb}_{j}")
                for kt in range(KT):
                    mm(ps[:], wv_sb[:, kt, :], xt[:, kt, :], "v",
                       start=(kt == 0), stop=(kt == KT - 1))
                vt = vtpool.tile([128, TBLK], f32, tag="vt", name=f"vt_{bb}_{j}")
                nc.vector.tensor_copy(vt[:], ps[:])
                for s4 in range(SPT):
                    sb_idx = (col0 // 128) + s4
                    pt = ps_tp.tile([128, 128], f32, tag="ps_tp", name=f"ptr_{bb}_{j}_{s4}")
                    vin = vt[:, s4 * 128:(s4 + 1) * 128]
                    iid = ident[:]
                    pout = pt[:]
                    if "tr" in rset:
                        vin = vin.bitcast(f32r)
                        iid = iid.bitcast(f32r)
                        pout = pout.bitcast(f32r)
                    nc.tensor.transpose(pout, vin, iid)
                    nc.vector.tensor_copy(
                        vaug[:, sb_idx, :].rearrange(
                            "p (g c) -> p g c", g=HPC)[:, :, 0:HD],
                        pt[:].rearrange("p (g c) -> p g c", g=HPC),
                    )

            def emit_attn(bb, j):
                col0 = bb * t + j * TBLK
                tsl = slice(col0, col0 + TBLK)
                n_i = (j + 1) * SPT
                avs = [ps_av.tile([65, TBLK], f32, tag="ps_av", name=f"av_{bb}_{j}_{h}")
                       for h in range(HPC)]

                def emit_av(i_, nh_pair):
                    for h in range(HPC):
                        sb_idx = (bb * t + i_ * 128) // 128
                        mm(avs[h][:], vaug[:, sb_idx, h * 66:h * 66 + HD + 1],
                           nh_pair[h][:], "av",
                           start=(i_ == 0), stop=(i_ == n_i - 1))

                nh_prev = None
                i_prev = -1
                for i in range(n_i):
                    ssl = slice(bb * t + i * 128, bb * t + i * 128 + 128)
                    dd = 128 * i - TBLK * j
                    nh_pair = []
                    for h in range(HPC):
                        hp = slice(h * HD, (h + 1) * HD)
                        ps = ps_s.tile([128, TBLK], f32, tag="ps_s",
                                       name=f"pss_{bb}_{j}_{i}_{h}")
                        mm(ps[:], kT_sb[hp, ssl], qT_sb[hp, tsl], "scores",
                           start=True, stop=True)
                        nh = npool.tile([128, TBLK], f32, tag="nh",
                                        name=f"nh_{bb}_{j}_{i}_{h}")
                        if dd < 0:
                            nc.scalar.activation(
                                nh[:], ps[:],
                                mybir.ActivationFunctionType.Exp, scale=0.125)
                        else:
                            if dd > 0:
                                nc.vector.memset(nh[:, 0:dd], 0.0)
                            nc.scalar.activation(
                                nh[:, dd:TBLK], ps[:, dd:TBLK],
                                mybir.ActivationFunctionType.Exp, scale=0.125)
                            nc.vector.tensor_mul(
                                nh[:, dd:dd + 128], nh[:, dd:dd + 128], mask[:])
                        nh_pair.append(nh)
                    # attn@v lags one i-step so exp (ACT) hides under PE
                    if nh_prev is not None:
                        emit_av(i_prev, nh_prev)
                    nh_prev, i_prev = nh_pair, i
                emit_av(i_prev, nh_prev)

                for h in range(HPC):
                    # reciprocal of Z at partition 64, then K=1 matmul
                    # broadcasts 1/Z across the 64 output partitions
                    rrow = zpool.tile([65, TBLK], f32, tag="rrow",
                                      name=f"rr_{bb}_{j}_{h}")
                    nc.vector.reciprocal(rrow[64:65, :], avs[h][64:65, :])
                    bc = ps_s.tile([HD, TBLK], f32, tag="ps_s", name=f"bc_{bb}_{j}_{h}")
                    mm(bc[:], ones1[64:65, :], rrow[64:65, :], "bcast",
                       start=True, stop=True)
                    # DVE may read only one PSUM operand: stage bc in SBUF
                    bcs = tmpool.tile([HD, TBLK], f32, tag="bcs", name=f"bcs_{bb}_{j}_{h}")
                    nc.scalar.copy(bcs[:], bc[:])
                    if h == 0:
                        nc.vector.tensor_mul(outT_sb[0:HD, tsl], avs[h][0:HD, :], bcs[:])
                    else:
                        tmp = tmpool.tile([HD, TBLK], f32, tag="tmp", name=f"tm_{bb}_{j}")
                        nc.vector.tensor_mul(tmp[:], avs[h][0:HD, :], bcs[:])
                        nc.gpsimd.dma_start(
                            out=outT_sb[h * HD:(h + 1) * HD, tsl], in_=tmp[:])

            def emit_proj(bb, j):
                col0 = bb * t + j * TBLK
                for tl in range(TBLK // 128):
                    tt = col0 // 128 + tl
                    for ib in range(NIB):
                        ps = ps_tp.tile([128, PW], f32, tag="ps_tp",
                                        name=f"psp_{bb}_{j}_{tl}_{ib}")
                        mm(ps[:], outT_sb[:, tt * 128:(tt + 1) * 128],
                           wp_sb[:, ib * PW:(ib + 1) * PW], "proj",
                           start=True, stop=True)
                        ot = opool.tile([128, PW], f32, tag="ot",
                                        name=f"ot_{bb}_{j}_{tl}_{ib}")
                        if (tl * NIB + ib) % 3 == 2:
                            nc.scalar.copy(ot[:], ps[:])
                        else:
                            nc.vector.tensor_copy(ot[:], ps[:])
                        nc.sync.dma_start(
                            out=out_p[tt * 128:(tt + 1) * 128, ib * PW:(ib + 1) * PW],
                            in_=ot[:])

            # software pipeline: QKV runs one t-block ahead of attention, and
            # the projection lags one block behind, so block-boundary DVE/DMA
            # latencies hide under attention PE work
            blocks = [(bb, j) for bb in range(b) for j in range(NJ)]
            emit_qkv(*blocks[0])
            for idx, blk in enumerate(blocks):
                if idx + 1 < len(blocks):
                    emit_qkv(*blocks[idx + 1])
                emit_attn(*blk)
                if idx >= 1:
                    emit_proj(*blocks[idx - 1])
            emit_proj(*blocks[-1])

    nc.compile()
    return nc


def _get_nc(b=B, t=T, d=D, cfg="f32"):
    key = (b, t, d, cfg)
    if key not in _CACHE:
        _CACHE[key] = _build(b, t, d, _CFGS[cfg])
    return _CACHE[key]


def _prepare_in_maps(x, Wq, Wk, Wv, Wp, b, t, d, n_heads):
    bt = b * t
    xT = np.ascontiguousarray(x.reshape(bt, d).T.astype(np.float32))
    in_maps = []
    for c in range(NCORES):
        h0 = c * HPC
        wq_c = np.ascontiguousarray(Wq[h0:h0 + HPC].reshape(CH, d).T.astype(np.float32))
        wk_c = np.ascontiguousarray(Wk[h0:h0 + HPC].reshape(CH, d).T.astype(np.float32))
        wv_c = np.ascontiguousarray(Wv[h0:h0 + HPC].reshape(CH, d).T.astype(np.float32))
        wp_c = np.ascontiguousarray(Wp[:, c * CH:(c + 1) * CH].T.astype(np.float32))
        in_maps.append({"xT": xT, "wq": wq_c, "wk": wk_c, "wv": wv_c, "wp": wp_c})
    return in_maps


def _run(x, Wq, Wk, Wv, Wp, bp, b, t, d, cfg, trace=False):
    from concourse.bass_utils import run_bass_kernel_spmd
    nc = _get_nc(b, t, d, cfg)
    in_maps = _prepare_in_maps(x, Wq, Wk, Wv, Wp, b, t, d, H)
    res = run_bass_kernel_spmd(nc, in_maps, core_ids=list(range(NCORES)), trace=trace)
    acc = np.zeros((b * t, d), dtype=np.float64)
    for r in res.results:
        acc += r["out_p"].astype(np.float64)
    out = (acc + np.asarray(bp, dtype=np.float64)).astype(np.float32)
    return out.reshape(b, t, d), res


KERNEL_CFG = "f32"


def kernel(x, Wq, Wk, Wv, Wp, bp):
    out, _ = _run(np.asarray(x), np.asarray(Wq), np.asarray(Wk), np.asarray(Wv),
                  np.asarray(Wp), np.asarray(bp), B, T, D, KERNEL_CFG, trace=False)
    return out



# revision 3
# speedup vs baseline: 2.7786x; 2.7786x over previous
"""Multi-head attention Trainium2 kernel (8 NeuronCores, tensor-parallel over heads).

Strategy:
  - 16 heads / 8 cores = 2 heads per core. x is replicated; Wq/Wk/Wv sharded by
    head; Wp row-sharded (contraction dim). Each core computes a partial
    projection output [B*T, D]; the host sums the 8 partials (+bias).
  - On chip, all contractions need the contracted dim on SBUF partitions, so the
    host passes xT = x.reshape(BT, D).T and per-core transposed weight slices.
  - qT/kT are computed packed [128 = 2 heads x 64, BT]. Scores are computed
    transposed (s on partitions, t on free) so softmax normalization can ride
    the attn@v matmul: lhsT = [v | ones] gives out rows 0..63 = unnormalized
    out^T and row 64 = the softmax denominator Z. Softmax is computed without
    max subtraction (scores are O(1), exp stays in fp32 range).
  - Causality: only lower-triangular [128s x 512t] blocks are computed; blocks
    straddling the diagonal are column-trimmed (scores/exp/attn@v only touch
    columns >= min(dd, 256)) and masked multiplicatively with a slice of a
    host-supplied shifted-staircase mask.
  - All matmul operands are float32r (TF32-like PE fast path, 4x f32 at free
    dim >= 256). The BIR verifier requires f32r operands to be *produced* as
    f32r, so every feeding tile is natively f32r: DMA'd inputs/constants are
    bitcast at the DMA, PSUM->SBUF copies and the exp write f32r directly.
"""

import numpy as np

B, T, D, H, HD = 2, 2048, 1024, 16, 64
NCORES = 8
HPC = H // NCORES          # heads per core = 2
CH = HPC * HD              # channels per core = 128
BT = B * T

_CACHE = {}


def _build(b, t, d, cfg):
    """Build + compile the per-core Bass program."""
    import concourse.tile as tile
    from concourse import bacc, mybir
    from contextlib import ExitStack

    f32 = mybir.dt.float32
    f32r = mybir.dt.float32r

    rmode = cfg == "r"
    MMDT = f32r if rmode else f32   # dtype of every matmul-feeding tile

    def bcst(ap):
        return ap.bitcast(f32r) if rmode else ap

    bt = b * t
    KT = d // 128            # k-tiles over the model dim
    TBLK = min(512, t)       # t-block width for scores/attn
    NJ = t // TBLK           # t-blocks per batch
    NSB = bt // 128          # 128-row s-blocks over B*T
    SPT = TBLK // 128        # s-blocks per t-block

    nc = bacc.Bacc("TRN2", target_bir_lowering=False, debug=False)

    xT = nc.dram_tensor("xT", [d, bt], f32, kind="ExternalInput").ap()
    wq = nc.dram_tensor("wq", [d, CH], f32, kind="ExternalInput").ap()
    wk = nc.dram_tensor("wk", [d, CH], f32, kind="ExternalInput").ap()
    wv = nc.dram_tensor("wv", [d, CH], f32, kind="ExternalInput").ap()
    wp = nc.dram_tensor("wp", [CH, d], f32, kind="ExternalInput").ap()
    cident = nc.dram_tensor("cident", [128, 128], f32, kind="ExternalInput").ap()
    cmask = nc.dram_tensor("cmask", [128, TBLK + 384], f32, kind="ExternalInput").ap()
    cones = nc.dram_tensor("cones", [128, NSB, HPC], f32, kind="ExternalInput").ap()
    cone1 = nc.dram_tensor("cone1", [65, HD], f32, kind="ExternalInput").ap()
    out_p = nc.dram_tensor("out_p", [bt, d], f32, kind="ExternalOutput").ap()

    with tile.TileContext(nc) as tc, ExitStack() as top:
        persist = top.enter_context(tc.tile_pool(name="persist", bufs=1))

        # ---- persistent tiles ----
        qT_sb = persist.tile([128, bt], MMDT, tag="qT")
        kT_sb = persist.tile([128, bt], MMDT, tag="kT")
        # [v_h0 | 1 | pad | v_h1 | 1 | pad] per 128-row s-block
        vaug = persist.tile([128, NSB, 66 * HPC], MMDT, tag="vaug")
        outT_sb = persist.tile([128, bt], MMDT, tag="outT")
        wq_sb = persist.tile([128, KT, CH], MMDT, tag="wq")
        wk_sb = persist.tile([128, KT, CH], MMDT, tag="wk")
        wv_sb = persist.tile([128, KT, CH], MMDT, tag="wv")
        wp_sb = persist.tile([128, d], MMDT, tag="wp")
        ident = persist.tile([128, 128], MMDT, tag="ident")
        # staircase mask, shifted: maskt[p, m] = 1 iff m >= p + 384
        maskt = persist.tile([128, TBLK + 384], f32, tag="mask")
        one1 = persist.tile([65, HD], MMDT, tag="one1")

        nc.gpsimd.dma_start(out=ident[:], in_=bcst(cident))
        nc.gpsimd.dma_start(out=maskt[:], in_=cmask)
        nc.gpsimd.dma_start(out=one1[:], in_=bcst(cone1))
        for h in range(HPC):
            nc.gpsimd.dma_start(
                out=vaug[:, :, 66 * h + 64:66 * h + 65],
                in_=bcst(cones[:, :, h:h + 1]),
            )
        for w_ap, w_sb in ((wq, wq_sb), (wk, wk_sb), (wv, wv_sb)):
            nc.gpsimd.dma_start(
                out=w_sb[:],
                in_=bcst(w_ap.rearrange("(kt p) m -> p kt m", p=128)),
            )
        nc.gpsimd.dma_start(out=wp_sb[:], in_=bcst(wp))

        # ---- merged loop: per (batch, t-block): QKV -> attention -> proj ----
        # Attention for block j of batch bb needs q columns of block j and
        # k/v columns of blocks 0..j (same batch) -- all computed by the time
        # block j's QKV is done, so one fused loop pipelines everything:
        # xT loads prefetch under attention PE work, and output stores drain
        # under the next block's compute.
        PW = min(512, d)
        NIB = d // PW
        with ExitStack() as body:
            xpool = body.enter_context(tc.tile_pool(name="xpool", bufs=3))
            vtpool = body.enter_context(tc.tile_pool(name="vtpool", bufs=2))
            npool = body.enter_context(tc.tile_pool(name="npool", bufs=8))
            zpool = body.enter_context(tc.tile_pool(name="zpool", bufs=2))
            tmpool = body.enter_context(tc.tile_pool(name="tmpool", bufs=2))
            opool = body.enter_context(tc.tile_pool(name="opool", bufs=4))
            # PSUM budget (8 banks): qkv 2 + scores/bcast 2 + av 2 + tr/proj 2
            ps_qkv = body.enter_context(tc.tile_pool(name="ps_qkv", bufs=2, space="PSUM"))
            ps_s = body.enter_context(tc.tile_pool(name="ps_s", bufs=2, space="PSUM"))
            ps_av = body.enter_context(tc.tile_pool(name="ps_av", bufs=2, space="PSUM"))
            ps_tp = body.enter_context(tc.tile_pool(name="ps_tp", bufs=2, space="PSUM"))

            def emit_qkv(bb, j):
                col0 = bb * t + j * TBLK
                tsl = slice(col0, col0 + TBLK)
                xt = xpool.tile([128, KT, TBLK], MMDT, tag="xt", name=f"xt_{bb}_{j}")
                for kt in range(KT):
                    nc.sync.dma_start(
                        out=xt[:, kt, :],
                        in_=bcst(xT[kt * 128:(kt + 1) * 128, tsl]),
                    )
                for w_sb, dst in ((wq_sb, qT_sb), (wk_sb, kT_sb)):
                    ps = ps_qkv.tile([128, TBLK], f32, tag="ps_qkv",
                                     name=f"psq_{bb}_{j}_{dst.name}")
                    for kt in range(KT):
                        nc.tensor.matmul(ps[:], w_sb[:, kt, :], xt[:, kt, :],
                                         start=(kt == 0), stop=(kt == KT - 1))
                    nc.vector.tensor_copy(dst[:, tsl], ps[:])
                ps = ps_qkv.tile([128, TBLK], f32, tag="ps_qkv", name=f"psv_{bb}_{j}")
                for kt in range(KT):
                    nc.tensor.matmul(ps[:], wv_sb[:, kt, :], xt[:, kt, :],
                                     start=(kt == 0), stop=(kt == KT - 1))
                vt = vtpool.tile([128, TBLK], MMDT, tag="vt", name=f"vt_{bb}_{j}")
                nc.vector.tensor_copy(vt[:], ps[:])
                for s4 in range(SPT):
                    sb_idx = (col0 // 128) + s4
                    pt = ps_tp.tile([128, 128], MMDT, tag="ps_tp",
                                    name=f"ptr_{bb}_{j}_{s4}")
                    nc.tensor.transpose(pt[:], vt[:, s4 * 128:(s4 + 1) * 128],
                                        ident[:])
                    nc.vector.tensor_copy(
                        vaug[:, sb_idx, :].rearrange(
                            "p (g c) -> p g c", g=HPC)[:, :, 0:HD],
                        pt[:].rearrange("p (g c) -> p g c", g=HPC),
                    )

            def emit_attn(bb, j):
                col0 = bb * t + j * TBLK
                tsl = slice(col0, col0 + TBLK)
                n_i = (j + 1) * SPT
                avs = [ps_av.tile([65, TBLK], f32, tag="ps_av", name=f"av_{bb}_{j}_{h}")
                       for h in range(HPC)]

                def emit_av(i_, ddp_, nh_pair):
                    for h in range(HPC):
                        sb_idx = (bb * t + i_ * 128) // 128
                        nc.tensor.matmul(
                            avs[h][:, ddp_:], vaug[:, sb_idx, h * 66:h * 66 + HD + 1],
                            nh_pair[h][:, ddp_:],
                            start=(i_ == 0), stop=(i_ == n_i - 1),
                            skip_group_check=True)

                nh_prev = None
                i_prev = -1
                ddp_prev = 0
                for i in range(n_i):
                    ssl = slice(bb * t + i * 128, bb * t + i * 128 + 128)
                    dd = 128 * i - TBLK * j
                    # column trim: scores/exp/av touch only cols >= ddp
                    # (f32r needs free dim >= 256 for the PE fast path)
                    ddp = max(0, min(dd, TBLK - 256))
                    nh_pair = []
                    for h in range(HPC):
                        hp = slice(h * HD, (h + 1) * HD)
                        ps = ps_s.tile([128, TBLK], f32, tag="ps_s",
                                       name=f"pss_{bb}_{j}_{i}_{h}")
                        nc.tensor.matmul(
                            ps[:, ddp:], kT_sb[hp, ssl],
                            qT_sb[hp, tsl][:, ddp:], start=True, stop=True)
                        nh = npool.tile([128, TBLK], MMDT, tag="nh",
                                        name=f"nh_{bb}_{j}_{i}_{h}")
                        nc.scalar.activation(
                            nh[:, ddp:], ps[:, ddp:],
                            mybir.ActivationFunctionType.Exp, scale=0.125)
                        if dd >= 0:
                            # mask cols [ddp, dd+128): staircase + trim slack
                            # (nh col c maps to mask col c + 384 - dd)
                            nc.vector.tensor_mul(
                                nh[:, ddp:dd + 128], nh[:, ddp:dd + 128],
                                maskt[:, 384 - dd + ddp:512])
                        nh_pair.append(nh)
                    # attn@v lags one i-step so exp (ACT) hides under PE
                    if nh_prev is not None:
                        emit_av(i_prev, ddp_prev, nh_prev)
                    nh_prev, i_prev, ddp_prev = nh_pair, i, ddp
                emit_av(i_prev, ddp_prev, nh_prev)

                # --- softmax normalization: out = av * (1/Z) ---
                rrow = zpool.tile([65, HPC * TBLK], f32, tag="rrow",
                                  name=f"rr_{bb}_{j}")
                for h in range(HPC):
                    nc.vector.reciprocal(
                        rrow[64:65, h * TBLK:(h + 1) * TBLK], avs[h][64:65, :])
                rr = zpool.tile([65, HPC * TBLK], MMDT, tag="rr",
                                name=f"rrr_{bb}_{j}")
                nc.vector.tensor_copy(rr[64:65, :], rrow[64:65, :])
                for h in range(HPC):
                    # K=1 matmul broadcasts 1/Z across the 64 output partitions
                    bc = ps_s.tile([HD, TBLK], f32, tag="ps_s", name=f"bc_{bb}_{j}_{h}")
                    nc.tensor.matmul(bc[:], one1[64:65, :],
                                     rr[64:65, h * TBLK:(h + 1) * TBLK],
                                     start=True, stop=True)
                    # DVE may read only one PSUM operand: stage bc in SBUF
                    bcs = tmpool.tile([HD, TBLK], f32, tag="bcs", name=f"bcs_{bb}_{j}_{h}")
                    nc.scalar.copy(bcs[:], bc[:])
                    if h == 0:
                        nc.vector.tensor_mul(outT_sb[0:HD, tsl], avs[h][0:HD, :], bcs[:])
                    else:
                        tmp = tmpool.tile([HD, TBLK], MMDT, tag="tmp", name=f"tm_{bb}_{j}")
                        nc.vector.tensor_mul(tmp[:], avs[h][0:HD, :], bcs[:])
                        nc.gpsimd.dma_start(
                            out=outT_sb[h * HD:(h + 1) * HD, tsl], in_=tmp[:])

            def emit_proj(bb, j):
                col0 = bb * t + j * TBLK
                for tl in range(TBLK // 128):
                    tt = col0 // 128 + tl
                    for ib in range(NIB):
                        ps = ps_tp.tile([128, PW], f32, tag="ps_tp",
                                        name=f"psp_{bb}_{j}_{tl}_{ib}")
                        nc.tensor.matmul(ps[:], outT_sb[:, tt * 128:(tt + 1) * 128],
                                         wp_sb[:, ib * PW:(ib + 1) * PW],
                                         start=True, stop=True)
                        ot = opool.tile([128, PW], f32, tag="ot",
                                        name=f"ot_{bb}_{j}_{tl}_{ib}")
                        if (tl * NIB + ib) % 3 == 2:
                            nc.scalar.copy(ot[:], ps[:])
                        else:
                            nc.vector.tensor_copy(ot[:], ps[:])
                        nc.sync.dma_start(
                            out=out_p[tt * 128:(tt + 1) * 128, ib * PW:(ib + 1) * PW],
                            in_=ot[:])

            # software pipeline: QKV runs one t-block ahead of attention, and
            # the projection lags one block behind, so block-boundary DVE/DMA
            # latencies hide under attention PE work
            blocks = [(bb, j) for bb in range(b) for j in range(NJ)]
            emit_qkv(*blocks[0])
            for idx, blk in enumerate(blocks):
                if idx + 1 < len(blocks):
                    emit_qkv(*blocks[idx + 1])
                emit_attn(*blk)
                if idx >= 1:
                    emit_proj(*blocks[idx - 1])
            emit_proj(*blocks[-1])

    nc.compile()
    return nc


def _get_nc(b=B, t=T, d=D, cfg="r"):
    key = (b, t, d, cfg)
    if key not in _CACHE:
        _CACHE[key] = _build(b, t, d, cfg)
    return _CACHE[key]


def _make_consts(b, t, d):
    bt = b * t
    TBLK = min(512, t)
    NSB = bt // 128
    cident = np.eye(128, dtype=np.float32)
    p = np.arange(128, dtype=np.int64)[:, None]
    m = np.arange(TBLK + 384, dtype=np.int64)[None, :]
    cmask = (m >= p + 384).astype(np.float32)
    cones = np.ones((128, NSB, HPC), dtype=np.float32)
    cone1 = np.ones((65, HD), dtype=np.float32)
    return {"cident": cident, "cmask": cmask, "cones": cones, "cone1": cone1}


def _prepare_in_maps(x, Wq, Wk, Wv, Wp, b, t, d):
    bt = b * t
    xT = np.ascontiguousarray(x.reshape(bt, d).T.astype(np.float32))
    consts = _make_consts(b, t, d)
    in_maps = []
    for c in range(NCORES):
        h0 = c * HPC
        wq_c = np.ascontiguousarray(Wq[h0:h0 + HPC].reshape(CH, d).T.astype(np.float32))
        wk_c = np.ascontiguousarray(Wk[h0:h0 + HPC].reshape(CH, d).T.astype(np.float32))
        wv_c = np.ascontiguousarray(Wv[h0:h0 + HPC].reshape(CH, d).T.astype(np.float32))
        wp_c = np.ascontiguousarray(Wp[:, c * CH:(c + 1) * CH].T.astype(np.float32))
        in_maps.append({"xT": xT, "wq": wq_c, "wk": wk_c, "wv": wv_c, "wp": wp_c,
                        **consts})
    return in_maps


def _run(x, Wq, Wk, Wv, Wp, bp, b, t, d, cfg, trace=False):
    from concourse.bass_utils import run_bass_kernel_spmd
    nc = _get_nc(b, t, d, cfg)
    in_maps = _prepare_in_maps(x, Wq, Wk, Wv, Wp, b, t, d)
    res = run_bass_kernel_spmd(nc, in_maps, core_ids=list(range(NCORES)), trace=trace)
    acc = np.zeros((b * t, d), dtype=np.float64)
    for r in res.results:
        acc += r["out_p"].astype(np.float64)
    out = (acc + np.asarray(bp, dtype=np.float64)).astype(np.float32)
    return out.reshape(b, t, d), res


KERNEL_CFG = "r"


def kernel(x, Wq, Wk, Wv, Wp, bp):
    out, _ = _run(np.asarray(x), np.asarray(Wq), np.asarray(Wk), np.asarray(Wv),
                  np.asarray(Wp), np.asarray(bp), B, T, D, KERNEL_CFG, trace=False)
    return out


# revision 12
# speedup vs baseline: 3.0300x; 1.0905x over previous
"""Multi-head attention Trainium2 kernel (8 NeuronCores, tensor-parallel over heads).

Strategy:
  - 16 heads / 8 cores = 2 heads per core. x is replicated; Wq/Wk/Wv sharded by
    head; Wp row-sharded (contraction dim). Each core computes a partial
    projection output [B*T, D]; the host sums the 8 partials (+bias).
  - On chip, all contractions need the contracted dim on SBUF partitions, so the
    host passes xT = x.reshape(BT, D).T and per-core transposed weight slices.
  - qT/kT are computed packed [128 = 2 heads x 64, BT]. Scores are computed
    transposed (s on partitions, t on free) so softmax normalization can ride
    the attn@v matmul: lhsT = [v | ones] gives out rows 0..63 = unnormalized
    out^T and row 64 = the softmax denominator Z. Softmax is computed without
    max subtraction (scores are O(1), exp stays in fp32 range).
  - Causality: only lower-triangular [128s x 512t] blocks are computed; blocks
    straddling the diagonal are column-trimmed (scores/exp/attn@v only touch
    columns >= min(dd, 256)) and masked multiplicatively with a slice of a
    host-supplied shifted-staircase mask.
  - All matmul operands are float32r (TF32-like PE fast path, 4x f32 at free
    dim >= 256). The BIR verifier requires f32r operands to be *produced* as
    f32r, so every feeding tile is natively f32r: DMA'd inputs/constants are
    bitcast at the DMA, PSUM->SBUF copies and the exp write f32r directly.
"""

import numpy as np

B, T, D, H, HD = 2, 2048, 1024, 16, 64
NCORES = 8
HPC = H // NCORES          # heads per core = 2
CH = HPC * HD              # channels per core = 128
BT = B * T

_CACHE = {}


def _build(b, t, d, cfg):
    """Build + compile the per-core Bass program."""
    import concourse.tile as tile
    from concourse import bacc, mybir
    from contextlib import ExitStack

    f32 = mybir.dt.float32
    f32r = mybir.dt.float32r
    bf16 = mybir.dt.bfloat16

    rmode = cfg == "r"
    bmode = cfg == "b"
    # dtype of every matmul-feeding tile
    MMDT = f32r if rmode else (bf16 if bmode else f32)
    # dtype of the DMA'd inputs (host converts for bf16)
    INDT = bf16 if bmode else f32

    def bcst(ap):
        return ap.bitcast(f32r) if rmode else ap

    bt = b * t
    KT = d // 128            # k-tiles over the model dim
    TBLK = min(512, t)       # t-block width for scores/attn
    NJ = t // TBLK           # t-blocks per batch
    NSB = bt // 128          # 128-row s-blocks over B*T
    SPT = TBLK // 128        # s-blocks per t-block

    nc = bacc.Bacc("TRN2", target_bir_lowering=False, debug=False)

    xT = nc.dram_tensor("xT", [d, bt], INDT, kind="ExternalInput").ap()
    wq = nc.dram_tensor("wq", [d, CH], INDT, kind="ExternalInput").ap()
    wk = nc.dram_tensor("wk", [d, CH], INDT, kind="ExternalInput").ap()
    wv = nc.dram_tensor("wv", [d, CH], INDT, kind="ExternalInput").ap()
    wp = nc.dram_tensor("wp", [CH, d], INDT, kind="ExternalInput").ap()
    cident = nc.dram_tensor("cident", [128, 128], INDT, kind="ExternalInput").ap()
    cmask = nc.dram_tensor("cmask", [128, TBLK + 384], INDT, kind="ExternalInput").ap()
    cones = nc.dram_tensor("cones", [128, NSB, HPC], INDT, kind="ExternalInput").ap()
    cone1 = nc.dram_tensor("cone1", [65, HD], INDT, kind="ExternalInput").ap()
    out_p = nc.dram_tensor("out_p", [bt, d], f32, kind="ExternalOutput").ap()

    with tile.TileContext(nc) as tc, ExitStack() as top:
        persist = top.enter_context(tc.tile_pool(name="persist", bufs=1))

        # ---- persistent tiles ----
        qT_sb = persist.tile([128, bt], MMDT, tag="qT")
        kT_sb = persist.tile([128, bt], MMDT, tag="kT")
        # [v_h0 | 1 | pad | v_h1 | 1 | pad] per 128-row s-block
        vaug = persist.tile([128, NSB, 66 * HPC], MMDT, tag="vaug")
        outT_sb = persist.tile([128, bt], MMDT, tag="outT")
        wq_sb = persist.tile([128, KT, CH], MMDT, tag="wq")
        wk_sb = persist.tile([128, KT, CH], MMDT, tag="wk")
        wv_sb = persist.tile([128, KT, CH], MMDT, tag="wv")
        wp_sb = persist.tile([128, d], MMDT, tag="wp")
        ident = persist.tile([128, 128], MMDT, tag="ident")
        # staircase mask, shifted: maskt[p, m] = 1 iff m >= p + 384
        maskt = persist.tile([128, TBLK + 384], MMDT, tag="mask")
        one1 = persist.tile([65, HD], MMDT, tag="one1")

        # startup DMAs ordered by first use: ident (block-0 transposes) and
        # per-kt weight chunks first so the first QKV matmul can start after
        # one 64KB chunk; attention consts next; wp (needed only by the
        # lagging proj stage) last.
        nc.gpsimd.dma_start(out=ident[:], in_=bcst(cident))
        # preload the Exp activation table under the startup DMAs
        actwarm = persist.tile([1, 8], f32, tag="actwarm")
        nc.scalar.activation(actwarm[:], ident[0:1, 0:8],
                             mybir.ActivationFunctionType.Exp, scale=0.125)
        for kt in range(KT):
            for w_ap, w_sb in ((wq, wq_sb), (wk, wk_sb), (wv, wv_sb)):
                nc.gpsimd.dma_start(
                    out=w_sb[:, kt, :],
                    in_=bcst(w_ap[kt * 128:(kt + 1) * 128, :]),
                )
        nc.gpsimd.dma_start(out=maskt[:], in_=cmask)
        nc.gpsimd.dma_start(out=one1[:], in_=bcst(cone1))
        for h in range(HPC):
            nc.gpsimd.dma_start(
                out=vaug[:, :, 66 * h + 64:66 * h + 65],
                in_=bcst(cones[:, :, h:h + 1]),
            )
        nc.gpsimd.dma_start(out=wp_sb[:], in_=bcst(wp))

        # ---- merged loop: per (batch, t-block): QKV -> attention -> proj ----
        # Attention for block j of batch bb needs q columns of block j and
        # k/v columns of blocks 0..j (same batch) -- all computed by the time
        # block j's QKV is done, so one fused loop pipelines everything:
        # xT loads prefetch under attention PE work, and output stores drain
        # under the next block's compute.
        PW = min(512, d)
        NIB = d // PW
        with ExitStack() as body:
            xpool = body.enter_context(tc.tile_pool(name="xpool", bufs=3))
            vtpool = body.enter_context(tc.tile_pool(name="vtpool", bufs=2))
            npool = body.enter_context(tc.tile_pool(name="npool", bufs=8))
            zpool = body.enter_context(tc.tile_pool(name="zpool", bufs=2))
            tmpool = body.enter_context(tc.tile_pool(name="tmpool", bufs=2))
            opool = body.enter_context(tc.tile_pool(name="opool", bufs=4))
            # PSUM budget (8 banks): qkv/tr/proj/bc 2 + scores 2x2 + av 2
            ps_qkv = body.enter_context(tc.tile_pool(name="ps_qkv", bufs=2, space="PSUM"))
            ps_s = body.enter_context(tc.tile_pool(name="ps_s", bufs=2, space="PSUM"))
            ps_av = body.enter_context(tc.tile_pool(name="ps_av", bufs=2, space="PSUM"))
            ps_tp = ps_qkv

            def emit_qkv(bb, j):
                col0 = bb * t + j * TBLK
                tsl = slice(col0, col0 + TBLK)
                xt = xpool.tile([128, KT, TBLK], MMDT, tag="xt", name=f"xt_{bb}_{j}")
                for kt in range(KT):
                    nc.sync.dma_start(
                        out=xt[:, kt, :],
                        in_=bcst(xT[kt * 128:(kt + 1) * 128, tsl]),
                    )
                for w_sb, dst in ((wq_sb, qT_sb), (wk_sb, kT_sb)):
                    ps = ps_qkv.tile([128, TBLK], f32, tag="ps_qkv",
                                     name=f"psq_{bb}_{j}_{dst.name}")
                    for kt in range(KT):
                        nc.tensor.matmul(ps[:], w_sb[:, kt, :], xt[:, kt, :],
                                         start=(kt == 0), stop=(kt == KT - 1))
                    nc.vector.tensor_copy(dst[:, tsl], ps[:])
                ps = ps_qkv.tile([128, TBLK], f32, tag="ps_qkv", name=f"psv_{bb}_{j}")
                for kt in range(KT):
                    nc.tensor.matmul(ps[:], wv_sb[:, kt, :], xt[:, kt, :],
                                     start=(kt == 0), stop=(kt == KT - 1))
                vt = vtpool.tile([128, TBLK], MMDT, tag="vt", name=f"vt_{bb}_{j}")
                nc.vector.tensor_copy(vt[:], ps[:])
                for s4 in range(SPT):
                    sb_idx = (col0 // 128) + s4
                    pt = ps_tp.tile([128, 128], MMDT, tag="ps_tp",
                                    name=f"ptr_{bb}_{j}_{s4}")
                    nc.tensor.transpose(pt[:], vt[:, s4 * 128:(s4 + 1) * 128],
                                        ident[:])
                    nc.vector.tensor_copy(
                        vaug[:, sb_idx, :].rearrange(
                            "p (g c) -> p g c", g=HPC)[:, :, 0:HD],
                        pt[:].rearrange("p (g c) -> p g c", g=HPC),
                    )

            def emit_attn(bb, j):
                col0 = bb * t + j * TBLK
                tsl = slice(col0, col0 + TBLK)
                n_i = (j + 1) * SPT
                avs = [ps_av.tile([65, TBLK], f32, tag="ps_av", name=f"av_{bb}_{j}_{h}")
                       for h in range(HPC)]

                def emit_av(i_, ddp_, nh_pair):
                    for h in range(HPC):
                        sb_idx = (bb * t + i_ * 128) // 128
                        nc.tensor.matmul(
                            avs[h][:, ddp_:], vaug[:, sb_idx, h * 66:h * 66 + HD + 1],
                            nh_pair[h][:, ddp_:],
                            start=(i_ == 0), stop=(i_ == n_i - 1),
                            skip_group_check=True)

                nh_prev = None
                i_prev = -1
                ddp_prev = 0
                for i in range(n_i):
                    ssl = slice(bb * t + i * 128, bb * t + i * 128 + 128)
                    dd = 128 * i - TBLK * j
                    # column trim: scores/exp/av touch only cols >= ddp
                    # (f32r needs free dim >= 256 for the PE fast path;
                    # bf16 runs 1 cyc/row at any width so trim fully)
                    if rmode:
                        ddp = max(0, min(dd, TBLK - 256))
                    else:
                        ddp = max(0, dd)
                    nh_pair = []
                    for h in range(HPC):
                        hp = slice(h * HD, (h + 1) * HD)
                        ps = ps_s.tile([128, TBLK], f32, tag="ps_s",
                                       name=f"pss_{bb}_{j}_{i}_{h}")
                        nc.tensor.matmul(
                            ps[:, ddp:], kT_sb[hp, ssl],
                            qT_sb[hp, tsl][:, ddp:], start=True, stop=True)
                        nh = npool.tile([128, TBLK], MMDT, tag="nh",
                                        name=f"nh_{bb}_{j}_{i}_{h}")
                        nc.scalar.activation(
                            nh[:, ddp:], ps[:, ddp:],
                            mybir.ActivationFunctionType.Exp, scale=0.125)
                        if dd >= 0:
                            # mask cols [ddp, dd+128): staircase + trim slack
                            # (nh col c maps to mask col c + 384 - dd)
                            nc.vector.tensor_mul(
                                nh[:, ddp:dd + 128], nh[:, ddp:dd + 128],
                                maskt[:, 384 - dd + ddp:512])
                        nh_pair.append(nh)
                    # attn@v lags one i-step so exp (ACT) hides under PE
                    if nh_prev is not None:
                        emit_av(i_prev, ddp_prev, nh_prev)
                    nh_prev, i_prev, ddp_prev = nh_pair, i, ddp
                emit_av(i_prev, ddp_prev, nh_prev)

                # --- softmax normalization: out = av * (1/Z) ---
                rr = zpool.tile([65, HPC * TBLK], MMDT, tag="rr",
                                name=f"rrr_{bb}_{j}")
                with nc.allow_low_precision(reason="f32r PE broadcast of 1/Z"):
                    for h in range(HPC):
                        nc.vector.reciprocal(
                            rr[64:65, h * TBLK:(h + 1) * TBLK], avs[h][64:65, :])
                for h in range(HPC):
                    # K=1 matmul broadcasts 1/Z across the 64 output partitions
                    bc = ps_tp.tile([HD, TBLK], f32, tag="ps_tp", name=f"bc_{bb}_{j}_{h}")
                    nc.tensor.matmul(bc[:], one1[64:65, :],
                                     rr[64:65, h * TBLK:(h + 1) * TBLK],
                                     start=True, stop=True)
                    # DVE may read only one PSUM operand: stage bc in SBUF
                    bcs = tmpool.tile([HD, TBLK], f32, tag="bcs", name=f"bcs_{bb}_{j}_{h}")
                    nc.scalar.copy(bcs[:], bc[:])
                    if h == 0:
                        nc.vector.tensor_mul(outT_sb[0:HD, tsl], avs[h][0:HD, :], bcs[:])
                    else:
                        tmp = tmpool.tile([HD, TBLK], MMDT, tag="tmp", name=f"tm_{bb}_{j}")
                        nc.vector.tensor_mul(tmp[:], avs[h][0:HD, :], bcs[:])
                        nc.gpsimd.dma_start(
                            out=outT_sb[h * HD:(h + 1) * HD, tsl], in_=tmp[:])

            def emit_proj(bb, j):
                col0 = bb * t + j * TBLK
                for tl in range(TBLK // 128):
                    tt = col0 // 128 + tl
                    for ib in range(NIB):
                        ps = ps_tp.tile([128, PW], f32, tag="ps_tp",
                                        name=f"psp_{bb}_{j}_{tl}_{ib}")
                        nc.tensor.matmul(ps[:], outT_sb[:, tt * 128:(tt + 1) * 128],
                                         wp_sb[:, ib * PW:(ib + 1) * PW],
                                         start=True, stop=True)
                        ot = opool.tile([128, PW], f32, tag="ot",
                                        name=f"ot_{bb}_{j}_{tl}_{ib}")
                        if (tl * NIB + ib) % 3 == 2:
                            nc.scalar.copy(ot[:], ps[:])
                        else:
                            nc.vector.tensor_copy(ot[:], ps[:])
                        nc.sync.dma_start(
                            out=out_p[tt * 128:(tt + 1) * 128, ib * PW:(ib + 1) * PW],
                            in_=ot[:])

            # software pipeline: QKV runs one t-block ahead of attention, and
            # the projection lags one block behind, so block-boundary DVE/DMA
            # latencies hide under attention PE work
            blocks = [(bb, j) for bb in range(b) for j in range(NJ)]
            emit_qkv(*blocks[0])
            for idx, blk in enumerate(blocks):
                if idx + 1 < len(blocks):
                    emit_qkv(*blocks[idx + 1])
                emit_attn(*blk)
                if idx >= 1:
                    emit_proj(*blocks[idx - 1])
            emit_proj(*blocks[-1])

    nc.compile()
    return nc


def _get_nc(b=B, t=T, d=D, cfg="r"):
    key = (b, t, d, cfg)
    if key not in _CACHE:
        _CACHE[key] = _build(b, t, d, cfg)
    return _CACHE[key]


def _in_dtype(cfg):
    if cfg == "b":
        import ml_dtypes
        return np.dtype(ml_dtypes.bfloat16)
    return np.dtype(np.float32)


def _make_consts(b, t, d, dt):
    bt = b * t
    TBLK = min(512, t)
    NSB = bt // 128
    cident = np.eye(128, dtype=dt)
    p = np.arange(128, dtype=np.int64)[:, None]
    m = np.arange(TBLK + 384, dtype=np.int64)[None, :]
    cmask = (m >= p + 384).astype(dt)
    cones = np.ones((128, NSB, HPC), dtype=dt)
    cone1 = np.ones((65, HD), dtype=dt)
    return {"cident": cident, "cmask": cmask, "cones": cones, "cone1": cone1}


def _prepare_in_maps(x, Wq, Wk, Wv, Wp, b, t, d, cfg):
    bt = b * t
    dt = _in_dtype(cfg)
    xT = np.ascontiguousarray(x.reshape(bt, d).T.astype(dt))
    consts = _make_consts(b, t, d, dt)
    in_maps = []
    for c in range(NCORES):
        h0 = c * HPC
        wq_c = np.ascontiguousarray(Wq[h0:h0 + HPC].reshape(CH, d).T.astype(dt))
        wk_c = np.ascontiguousarray(Wk[h0:h0 + HPC].reshape(CH, d).T.astype(dt))
        wv_c = np.ascontiguousarray(Wv[h0:h0 + HPC].reshape(CH, d).T.astype(dt))
        wp_c = np.ascontiguousarray(Wp[:, c * CH:(c + 1) * CH].T.astype(dt))
        in_maps.append({"xT": xT, "wq": wq_c, "wk": wk_c, "wv": wv_c, "wp": wp_c,
                        **consts})
    return in_maps


def _run(x, Wq, Wk, Wv, Wp, bp, b, t, d, cfg, trace=False):
    from concourse.bass_utils import run_bass_kernel_spmd
    nc = _get_nc(b, t, d, cfg)
    in_maps = _prepare_in_maps(x, Wq, Wk, Wv, Wp, b, t, d, cfg)
    res = run_bass_kernel_spmd(nc, in_maps, core_ids=list(range(NCORES)), trace=trace)
    acc = np.zeros((b * t, d), dtype=np.float64)
    for r in res.results:
        acc += r["out_p"].astype(np.float64)
    out = (acc + np.asarray(bp, dtype=np.float64)).astype(np.float32)
    return out.reshape(b, t, d), res


KERNEL_CFG = "r"


def kernel(x, Wq, Wk, Wv, Wp, bp):
    out, _ = _run(np.asarray(x), np.asarray(Wq), np.asarray(Wk), np.asarray(Wv),
                  np.asarray(Wp), np.asarray(bp), B, T, D, KERNEL_CFG, trace=False)
    return out


# revision 23
# speedup vs baseline: 3.3606x; 1.1091x over previous
"""Multi-head attention Trainium2 kernel (8 NeuronCores, tensor-parallel over heads).

Strategy:
  - 16 heads / 8 cores = 2 heads per core. x is replicated; Wq/Wk/Wv sharded by
    head; Wp row-sharded (contraction dim). Each core computes a partial
    projection output [B*T, D]; the host sums the 8 partials (+bias).
  - On chip, all contractions need the contracted dim on SBUF partitions, so the
    host passes xT = x.reshape(BT, D).T and per-core transposed weight slices.
  - qT/kT are computed packed [128 = 2 heads x 64, BT]. Scores are computed
    transposed (s on partitions, t on free) so softmax normalization can ride
    the attn@v matmul: lhsT = [v | ones] gives out rows 0..63 = unnormalized
    out^T and row 64 = the softmax denominator Z. Softmax is computed without
    max subtraction (scores are O(1), exp stays in fp32 range).
  - Causality: only lower-triangular [128s x 512t] blocks are computed; blocks
    straddling the diagonal are column-trimmed (scores/exp/attn@v only touch
    columns >= min(dd, 256)) and masked multiplicatively with a slice of a
    host-supplied shifted-staircase mask.
  - All matmul operands are float32r (TF32-like PE fast path, 4x f32 at free
    dim >= 256). The BIR verifier requires f32r operands to be *produced* as
    f32r, so every feeding tile is natively f32r: DMA'd inputs/constants are
    bitcast at the DMA, PSUM->SBUF copies and the exp write f32r directly.
"""

import numpy as np

B, T, D, H, HD = 2, 2048, 1024, 16, 64
NCORES = 8
HPC = H // NCORES          # heads per core = 2
CH = HPC * HD              # channels per core = 128
BT = B * T

_CACHE = {}


def _build(b, t, d, cfg):
    """Build + compile the per-core Bass program."""
    import concourse.tile as tile
    from concourse import bacc, mybir
    from contextlib import ExitStack

    f32 = mybir.dt.float32
    f32r = mybir.dt.float32r
    bf16 = mybir.dt.bfloat16

    rmode = cfg == "r"
    bmode = cfg == "b"
    # dtype of every matmul-feeding tile
    MMDT = f32r if rmode else (bf16 if bmode else f32)
    # dtype of the DMA'd inputs (host converts for bf16)
    INDT = bf16 if bmode else f32

    def bcst(ap):
        return ap.bitcast(f32r) if rmode else ap

    bt = b * t
    KT = d // 128            # k-tiles over the model dim
    TBLK = min(512, t)       # t-block width for scores/attn
    NJ = t // TBLK           # t-blocks per batch
    NSB = bt // 128          # 128-row s-blocks over B*T
    SPT = TBLK // 128        # s-blocks per t-block

    nc = bacc.Bacc("TRN2", target_bir_lowering=False, debug=False)

    xT = nc.dram_tensor("xT", [d, bt], INDT, kind="ExternalInput").ap()
    wq = nc.dram_tensor("wq", [d, CH], INDT, kind="ExternalInput").ap()
    wk = nc.dram_tensor("wk", [d, CH], INDT, kind="ExternalInput").ap()
    wv = nc.dram_tensor("wv", [d, CH], INDT, kind="ExternalInput").ap()
    wp = nc.dram_tensor("wp", [CH, d], INDT, kind="ExternalInput").ap()
    cident = nc.dram_tensor("cident", [128, 128], INDT, kind="ExternalInput").ap()
    cmask = nc.dram_tensor("cmask", [128, TBLK + 384], INDT, kind="ExternalInput").ap()
    cones = nc.dram_tensor("cones", [128, NSB, HPC], INDT, kind="ExternalInput").ap()
    cone1 = nc.dram_tensor("cone1", [65, HD], INDT, kind="ExternalInput").ap()
    # partials are summed across cores on the host in f64; bf16 partial
    # stores halve the output DMA traffic for ~1e-3 extra absmax-rel error
    OUTDT = bf16 if bmode else f32
    out_p = nc.dram_tensor("out_p", [bt, d], OUTDT, kind="ExternalOutput").ap()

    with tile.TileContext(nc) as tc, ExitStack() as top:
        persist = top.enter_context(tc.tile_pool(name="persist", bufs=1))

        # ---- persistent tiles ----
        qT_sb = persist.tile([128, bt], MMDT, tag="qT")
        kT_sb = persist.tile([128, bt], MMDT, tag="kT")
        # [v_h0 | 1 | pad | v_h1 | 1 | pad] per 128-row s-block
        vaug = persist.tile([128, NSB, 66 * HPC], MMDT, tag="vaug")
        outT_sb = persist.tile([128, bt], MMDT, tag="outT")
        wq_sb = persist.tile([128, KT, CH], MMDT, tag="wq")
        wk_sb = persist.tile([128, KT, CH], MMDT, tag="wk")
        wv_sb = persist.tile([128, KT, CH], MMDT, tag="wv")
        wp_sb = persist.tile([128, d], MMDT, tag="wp")
        ident = persist.tile([128, 128], MMDT, tag="ident")
        # staircase mask, shifted: maskt[p, m] = 1 iff m >= p + 384
        maskt = persist.tile([128, TBLK + 384], MMDT, tag="mask")
        one1 = persist.tile([65, HD], MMDT, tag="one1")

        # startup DMAs on the scalar HWDGE queue (fast descriptor gen; the
        # Pool SWDGE takes ~1.1us per DMA), ordered by first use: ident
        # (act-table warm + block-0 transposes), big wq/wk/wv loads, then
        # attention consts; wp (needed only by the lagging proj) last.
        for w_ap, w_sb in ((wq, wq_sb), (wk, wk_sb), (wv, wv_sb)):
            nc.scalar.dma_start(
                out=w_sb[:],
                in_=bcst(w_ap.rearrange("(kt p) m -> p kt m", p=128)),
            )
        # preload the Exp activation table under the startup DMAs
        actwarm = persist.tile([1, 8], f32, tag="actwarm")
        nc.scalar.activation(actwarm[:], wq_sb[0:1, 0, 0:8],
                             mybir.ActivationFunctionType.Exp, scale=0.125)
        nc.scalar.dma_start(out=ident[:], in_=bcst(cident))
        nc.scalar.dma_start(out=maskt[:], in_=cmask)
        nc.scalar.dma_start(out=one1[:], in_=bcst(cone1))
        for h in range(HPC):
            nc.scalar.dma_start(
                out=vaug[:, :, 66 * h + 64:66 * h + 65],
                in_=bcst(cones[:, :, h:h + 1]),
            )
        nc.scalar.dma_start(out=wp_sb[:], in_=bcst(wp))

        # ---- merged loop: per (batch, t-block): QKV -> attention -> proj ----
        # Attention for block j of batch bb needs q columns of block j and
        # k/v columns of blocks 0..j (same batch) -- all computed by the time
        # block j's QKV is done, so one fused loop pipelines everything:
        # xT loads prefetch under attention PE work, and output stores drain
        # under the next block's compute.
        PW = min(512, d)
        NIB = d // PW
        with ExitStack() as body:
            xpool = body.enter_context(tc.tile_pool(name="xpool", bufs=4))
            vtpool = body.enter_context(tc.tile_pool(name="vtpool", bufs=2))
            npool = body.enter_context(tc.tile_pool(name="npool", bufs=8))
            zpool = body.enter_context(tc.tile_pool(name="zpool", bufs=2))
            tmpool = body.enter_context(tc.tile_pool(name="tmpool", bufs=2))
            opool = body.enter_context(tc.tile_pool(name="opool", bufs=2))
            # PSUM budget (8 banks): qkv 2 + scores 2 + av 2 + tr/proj/bc 2
            ps_qkv = body.enter_context(tc.tile_pool(name="ps_qkv", bufs=2, space="PSUM"))
            ps_s = body.enter_context(tc.tile_pool(name="ps_s", bufs=2, space="PSUM"))
            ps_av = body.enter_context(tc.tile_pool(name="ps_av", bufs=2, space="PSUM"))
            ps_tp = body.enter_context(tc.tile_pool(name="ps_tp", bufs=2, space="PSUM"))

            xt_tiles = {}

            def emit_xt(bb, j, chunked=False):
                col0 = bb * t + j * TBLK
                tsl = slice(col0, col0 + TBLK)
                xt = xpool.tile([128, KT, TBLK], MMDT, tag="xt", name=f"xt_{bb}_{j}")
                if chunked:
                    for kt in range(KT):
                        nc.sync.dma_start(
                            out=xt[:, kt, :],
                            in_=bcst(xT[kt * 128:(kt + 1) * 128, tsl]),
                        )
                else:
                    nc.sync.dma_start(
                        out=xt[:],
                        in_=bcst(xT.rearrange("(kt p) c -> p kt c", p=128)[:, :, tsl]),
                    )
                xt_tiles[(bb, j)] = xt

            def emit_qkv(bb, j):
                col0 = bb * t + j * TBLK
                tsl = slice(col0, col0 + TBLK)
                xt = xt_tiles.pop((bb, j))
                for w_sb, dst in ((wq_sb, qT_sb), (wk_sb, kT_sb)):
                    ps = ps_qkv.tile([128, TBLK], f32, tag="ps_qkv",
                                     name=f"psq_{bb}_{j}_{dst.name}")
                    for kt in range(KT):
                        nc.tensor.matmul(ps[:], w_sb[:, kt, :], xt[:, kt, :],
                                         start=(kt == 0), stop=(kt == KT - 1))
                    nc.vector.tensor_copy(dst[:, tsl], ps[:])
                ps = ps_qkv.tile([128, TBLK], f32, tag="ps_qkv", name=f"psv_{bb}_{j}")
                for kt in range(KT):
                    nc.tensor.matmul(ps[:], wv_sb[:, kt, :], xt[:, kt, :],
                                     start=(kt == 0), stop=(kt == KT - 1))
                vt = vtpool.tile([128, TBLK], MMDT, tag="vt", name=f"vt_{bb}_{j}")
                nc.vector.tensor_copy(vt[:], ps[:])
                for s4 in range(SPT):
                    sb_idx = (col0 // 128) + s4
                    pt = ps_tp.tile([128, 128], MMDT, tag="ps_tp",
                                    name=f"ptr_{bb}_{j}_{s4}")
                    nc.tensor.transpose(pt[:], vt[:, s4 * 128:(s4 + 1) * 128],
                                        ident[:])
                    nc.vector.tensor_copy(
                        vaug[:, sb_idx, :].rearrange(
                            "p (g c) -> p g c", g=HPC)[:, :, 0:HD],
                        pt[:].rearrange("p (g c) -> p g c", g=HPC),
                    )

            def emit_attn(bb, j):
                col0 = bb * t + j * TBLK
                tsl = slice(col0, col0 + TBLK)
                n_i = (j + 1) * SPT
                avs = [ps_av.tile([65, TBLK], f32, tag="ps_av", name=f"av_{bb}_{j}_{h}")
                       for h in range(HPC)]

                def emit_av(i_, ddp_, nh_):
                    for h in range(HPC):
                        sb_idx = (bb * t + i_ * 128) // 128
                        nc.tensor.matmul(
                            avs[h][:, ddp_:], vaug[:, sb_idx, h * 66:h * 66 + HD + 1],
                            nh_[:, h * TBLK + ddp_:(h + 1) * TBLK],
                            start=(i_ == 0), stop=(i_ == n_i - 1),
                            skip_group_check=True)

                nh_prev = None
                i_prev = -1
                ddp_prev = 0
                for i in range(n_i):
                    ssl = slice(bb * t + i * 128, bb * t + i * 128 + 128)
                    dd = 128 * i - TBLK * j
                    # column trim: scores/exp/av touch only cols >= ddp
                    # (f32r needs free dim >= 256 for the PE fast path;
                    # bf16 runs 1 cyc/row at any width so trim fully)
                    if rmode:
                        ddp = max(0, min(dd, TBLK - 256))
                    else:
                        ddp = max(0, dd)
                    nh = npool.tile([128, HPC * TBLK], MMDT, tag="nh",
                                    name=f"nh_{bb}_{j}_{i}")
                    for h in range(HPC):
                        hp = slice(h * HD, (h + 1) * HD)
                        ps = ps_s.tile([128, TBLK], f32, tag="ps_s",
                                       name=f"pss_{bb}_{j}_{i}_{h}")
                        nc.tensor.matmul(
                            ps[:, ddp:], kT_sb[hp, ssl],
                            qT_sb[hp, tsl][:, ddp:], start=True, stop=True)
                        nc.scalar.activation(
                            nh[:, h * TBLK + ddp:(h + 1) * TBLK], ps[:, ddp:],
                            mybir.ActivationFunctionType.Exp, scale=0.125)
                        if dd >= 0:
                            # mask cols [ddp, dd+128): staircase + trim slack
                            # (nh col c maps to mask col c + 384 - dd)
                            nc.gpsimd.tensor_mul(
                                nh[:, h * TBLK + ddp:h * TBLK + dd + 128],
                                nh[:, h * TBLK + ddp:h * TBLK + dd + 128],
                                maskt[:, 384 - dd + ddp:512])
                    # attn@v lags one i-step so exp (ACT) hides under PE
                    if nh_prev is not None:
                        emit_av(i_prev, ddp_prev, nh_prev)
                    nh_prev, i_prev, ddp_prev = nh, i, ddp
                emit_av(i_prev, ddp_prev, nh_prev)

                # --- softmax normalization: out = av * (1/Z) ---
                rr = zpool.tile([65, HPC * TBLK], MMDT, tag="rr",
                                name=f"rrr_{bb}_{j}")
                with nc.allow_low_precision(reason="f32r PE broadcast of 1/Z"):
                    for h in range(HPC):
                        nc.vector.reciprocal(
                            rr[64:65, h * TBLK:(h + 1) * TBLK], avs[h][64:65, :])
                for h in range(HPC):
                    # K=1 matmul broadcasts 1/Z across the 64 output partitions
                    bc = ps_tp.tile([HD, TBLK], f32, tag="ps_tp", name=f"bc_{bb}_{j}_{h}")
                    nc.tensor.matmul(bc[:], one1[64:65, :],
                                     rr[64:65, h * TBLK:(h + 1) * TBLK],
                                     start=True, stop=True)
                    # DVE may read only one PSUM operand: stage bc in SBUF
                    # (on DVE -- ACT is loaded with the exp stream)
                    bcs = tmpool.tile([HD, TBLK], f32, tag="bcs", name=f"bcs_{bb}_{j}_{h}")
                    nc.vector.tensor_copy(bcs[:], bc[:])
                    if h == 0:
                        nc.vector.tensor_mul(outT_sb[0:HD, tsl], avs[h][0:HD, :], bcs[:])
                    else:
                        tmp = tmpool.tile([HD, TBLK], MMDT, tag="tmp", name=f"tm_{bb}_{j}")
                        nc.vector.tensor_mul(tmp[:], avs[h][0:HD, :], bcs[:])
                        nc.sync.dma_start(
                            out=outT_sb[h * HD:(h + 1) * HD, tsl], in_=tmp[:])

            def emit_proj(bb, j, last=False):
                col0 = bb * t + j * TBLK
                NTL = TBLK // 128
                ot = opool.tile([128, NTL, d], OUTDT, tag="ot", name=f"ot_{bb}_{j}")
                for tl in range(NTL):
                    tt = col0 // 128 + tl
                    for ib in range(NIB):
                        ps = ps_tp.tile([128, PW], f32, tag="ps_tp",
                                        name=f"psp_{bb}_{j}_{tl}_{ib}")
                        nc.tensor.matmul(ps[:], outT_sb[:, tt * 128:(tt + 1) * 128],
                                         wp_sb[:, ib * PW:(ib + 1) * PW],
                                         start=True, stop=True)
                        # in the drain there is no exp stream: split copies
                        # between DVE and ACT and store per row-block pair so
                        # the store overlaps the remaining copies
                        if last and (tl * NIB + ib) % 2 == 1:
                            nc.scalar.copy(ot[:, tl, ib * PW:(ib + 1) * PW], ps[:])
                        else:
                            nc.vector.tensor_copy(
                                ot[:, tl, ib * PW:(ib + 1) * PW], ps[:])
                    if last and tl % 2 == 1:
                        nc.sync.dma_start(
                            out=out_p.rearrange("(tb p) c -> p tb c", p=128)[
                                :, col0 // 128 + tl - 1:col0 // 128 + tl + 1, :],
                            in_=ot[:, tl - 1:tl + 1, :])
                if not last:
                    # one store DMA per block: [p, tl, d] -> row-blocks of out_p
                    nc.sync.dma_start(
                        out=out_p.rearrange("(tb p) c -> p tb c", p=128)[
                            :, col0 // 128:col0 // 128 + NTL, :],
                        in_=ot[:])

            # software pipeline: QKV runs one t-block ahead of attention, and
            # the projection lags one block behind, so block-boundary DVE/DMA
            # latencies hide under attention PE work
            blocks = [(bb, j) for bb in range(b) for j in range(NJ)]
            emit_xt(*blocks[0], chunked=True)
            emit_xt(*blocks[1])
            emit_qkv(*blocks[0])
            for idx, blk in enumerate(blocks):
                if idx + 2 < len(blocks):
                    emit_xt(*blocks[idx + 2])
                if idx + 1 < len(blocks):
                    emit_qkv(*blocks[idx + 1])
                emit_attn(*blk)
                if idx >= 1:
                    emit_proj(*blocks[idx - 1], last=(idx == len(blocks) - 1))
            emit_proj(*blocks[-1], last=True)

    nc.compile()
    return nc


def _get_nc(b=B, t=T, d=D, cfg="r"):
    key = (b, t, d, cfg)
    if key not in _CACHE:
        _CACHE[key] = _build(b, t, d, cfg)
    return _CACHE[key]


def _in_dtype(cfg):
    if cfg == "b":
        import ml_dtypes
        return np.dtype(ml_dtypes.bfloat16)
    return np.dtype(np.float32)


def _make_consts(b, t, d, dt):
    bt = b * t
    TBLK = min(512, t)
    NSB = bt // 128
    cident = np.eye(128, dtype=dt)
    p = np.arange(128, dtype=np.int64)[:, None]
    m = np.arange(TBLK + 384, dtype=np.int64)[None, :]
    cmask = (m >= p + 384).astype(dt)
    cones = np.ones((128, NSB, HPC), dtype=dt)
    cone1 = np.ones((65, HD), dtype=dt)
    return {"cident": cident, "cmask": cmask, "cones": cones, "cone1": cone1}


def _prepare_in_maps(x, Wq, Wk, Wv, Wp, b, t, d, cfg):
    bt = b * t
    dt = _in_dtype(cfg)
    xT = np.ascontiguousarray(x.reshape(bt, d).T.astype(dt))
    consts = _make_consts(b, t, d, dt)
    in_maps = []
    for c in range(NCORES):
        h0 = c * HPC
        wq_c = np.ascontiguousarray(Wq[h0:h0 + HPC].reshape(CH, d).T.astype(dt))
        wk_c = np.ascontiguousarray(Wk[h0:h0 + HPC].reshape(CH, d).T.astype(dt))
        wv_c = np.ascontiguousarray(Wv[h0:h0 + HPC].reshape(CH, d).T.astype(dt))
        wp_c = np.ascontiguousarray(Wp[:, c * CH:(c + 1) * CH].T.astype(dt))
        in_maps.append({"xT": xT, "wq": wq_c, "wk": wk_c, "wv": wv_c, "wp": wp_c,
                        **consts})
    return in_maps


def _run(x, Wq, Wk, Wv, Wp, bp, b, t, d, cfg, trace=False):
    from concourse.bass_utils import run_bass_kernel_spmd
    nc = _get_nc(b, t, d, cfg)
    in_maps = _prepare_in_maps(x, Wq, Wk, Wv, Wp, b, t, d, cfg)
    res = run_bass_kernel_spmd(nc, in_maps, core_ids=list(range(NCORES)), trace=trace)
    acc = np.zeros((b * t, d), dtype=np.float64)
    for r in res.results:
        acc += r["out_p"].astype(np.float64)
    out = (acc + np.asarray(bp, dtype=np.float64)).astype(np.float32)
    return out.reshape(b, t, d), res


KERNEL_CFG = "r"


def kernel(x, Wq, Wk, Wv, Wp, bp):
    out, _ = _run(np.asarray(x), np.asarray(Wq), np.asarray(Wk), np.asarray(Wv),
                  np.asarray(Wp), np.asarray(bp), B, T, D, KERNEL_CFG, trace=False)
    return out


# revision 51
# speedup vs baseline: 3.5119x; 1.0450x over previous
"""Multi-head attention Trainium2 kernel (8 NeuronCores, tensor-parallel over heads).

Strategy:
  - 16 heads / 8 cores = 2 heads per core. x is replicated; Wq/Wk/Wv sharded by
    head; Wp row-sharded (contraction dim). Each core computes a partial
    projection output [B*T, D]; the host sums the 8 partials (+bias).
  - On chip, all contractions need the contracted dim on SBUF partitions, so the
    host passes xT = x.reshape(BT, D).T and per-core transposed weight slices.
  - qT/kT are computed packed [128 = 2 heads x 64, BT]. Scores are computed
    transposed (s on partitions, t on free) so softmax normalization can ride
    the attn@v matmul: lhsT = [v | ones] gives out rows 0..63 = unnormalized
    out^T and row 64 = the softmax denominator Z. Softmax is computed without
    max subtraction (scores are O(1), exp stays in fp32 range).
  - Causality: only lower-triangular [128s x 512t] blocks are computed; blocks
    straddling the diagonal are column-trimmed (scores/exp/attn@v only touch
    columns >= min(dd, 256)) and masked multiplicatively with a slice of a
    host-supplied shifted-staircase mask.
  - All matmul operands are float32r (TF32-like PE fast path, 4x f32 at free
    dim >= 256). The BIR verifier requires f32r operands to be *produced* as
    f32r, so every feeding tile is natively f32r: DMA'd inputs/constants are
    bitcast at the DMA, PSUM->SBUF copies and the exp write f32r directly.
"""

import numpy as np

B, T, D, H, HD = 2, 2048, 1024, 16, 64
NCORES = 8
HPC = H // NCORES          # heads per core = 2
CH = HPC * HD              # channels per core = 128
BT = B * T

_CACHE = {}


def _build(b, t, d, cfg):
    """Build + compile the per-core Bass program."""
    import concourse.tile as tile
    from concourse import bacc, mybir
    from contextlib import ExitStack

    f32 = mybir.dt.float32
    f32r = mybir.dt.float32r
    bf16 = mybir.dt.bfloat16

    rmode = cfg == "r"
    bmode = cfg == "b"
    # dtype of every matmul-feeding tile
    MMDT = f32r if rmode else (bf16 if bmode else f32)
    # dtype of the DMA'd inputs (host converts for bf16)
    INDT = bf16 if bmode else f32

    def bcst(ap):
        return ap.bitcast(f32r) if rmode else ap

    bt = b * t
    KT = d // 128            # k-tiles over the model dim
    TBLK = min(512, t)       # t-block width for scores/attn
    NJ = t // TBLK           # t-blocks per batch
    NSB = bt // 128          # 128-row s-blocks over B*T
    SPT = TBLK // 128        # s-blocks per t-block

    nc = bacc.Bacc("TRN2", target_bir_lowering=False, debug=False)

    xT = nc.dram_tensor("xT", [d, bt], INDT, kind="ExternalInput").ap()
    wq = nc.dram_tensor("wq", [d, CH], INDT, kind="ExternalInput").ap()
    wk = nc.dram_tensor("wk", [d, CH], INDT, kind="ExternalInput").ap()
    wv = nc.dram_tensor("wv", [d, CH], INDT, kind="ExternalInput").ap()
    wp = nc.dram_tensor("wp", [CH, d], INDT, kind="ExternalInput").ap()
    cident = nc.dram_tensor("cident", [128, 128], INDT, kind="ExternalInput").ap()
    cmask = nc.dram_tensor("cmask", [128, TBLK + 384], INDT, kind="ExternalInput").ap()
    cones = nc.dram_tensor("cones", [128, NSB, HPC], INDT, kind="ExternalInput").ap()
    cone1 = nc.dram_tensor("cone1", [65, HD], INDT, kind="ExternalInput").ap()
    # partials are summed across cores on the host in f64; bf16 partial
    # stores halve the output DMA traffic for ~1e-3 extra absmax-rel error
    OUTDT = bf16 if bmode else f32
    out_p = nc.dram_tensor("out_p", [bt, d], OUTDT, kind="ExternalOutput").ap()

    with tile.TileContext(nc) as tc, ExitStack() as top:
        persist = top.enter_context(tc.tile_pool(name="persist", bufs=1))

        # ---- persistent tiles ----
        qT_sb = persist.tile([128, bt], MMDT, tag="qT")
        kT_sb = persist.tile([128, bt], MMDT, tag="kT")
        # [v_h0 | 1 | pad | v_h1 | 1 | pad] per 128-row s-block
        vaug = persist.tile([128, NSB, 66 * HPC], MMDT, tag="vaug")
        outT_sb = persist.tile([128, bt], MMDT, tag="outT")
        wq_sb = persist.tile([128, KT, CH], MMDT, tag="wq")
        wk_sb = persist.tile([128, KT, CH], MMDT, tag="wk")
        wv_sb = persist.tile([128, KT, CH], MMDT, tag="wv")
        wp_sb = persist.tile([128, d], MMDT, tag="wp")
        ident = persist.tile([128, 128], MMDT, tag="ident")
        # staircase mask, shifted: maskt[p, m] = 1 iff m >= p + 384
        maskt = persist.tile([128, TBLK + 384], MMDT, tag="mask")
        one1 = persist.tile([65, HD], MMDT, tag="one1")

        # startup DMAs on the scalar HWDGE queue (fast descriptor gen; the
        # Pool SWDGE takes ~1.1us per DMA), ordered by first use: ident
        # (act-table warm + block-0 transposes), big wq/wk/wv loads, then
        # attention consts; wp (needed only by the lagging proj) last.
        # wq in two halves so the first q matmuls start ~1.4us earlier
        for w_ap, w_sb, nsplit in ((wq, wq_sb, 2), (wk, wk_sb, 1), (wv, wv_sb, 1)):
            for s in range(nsplit):
                hk = slice(s * KT // nsplit, (s + 1) * KT // nsplit)
                nc.scalar.dma_start(
                    out=w_sb[:, hk, :],
                    in_=bcst(w_ap.rearrange("(kt p) m -> p kt m", p=128)[:, hk, :]),
                )
        # preload the Exp activation table under the startup DMAs
        actwarm = persist.tile([1, 8], f32, tag="actwarm")
        nc.scalar.activation(actwarm[:], wq_sb[0:1, 0, 0:8],
                             mybir.ActivationFunctionType.Exp, scale=0.125)
        nc.scalar.dma_start(out=ident[:], in_=bcst(cident))
        nc.scalar.dma_start(out=maskt[:], in_=bcst(cmask))
        nc.scalar.dma_start(out=one1[:], in_=bcst(cone1))
        for h in range(HPC):
            nc.scalar.dma_start(
                out=vaug[:, :, 66 * h + 64:66 * h + 65],
                in_=bcst(cones[:, :, h:h + 1]),
            )
        nc.scalar.dma_start(out=wp_sb[:], in_=bcst(wp))

        # ---- merged loop: per (batch, t-block): QKV -> attention -> proj ----
        # Attention for block j of batch bb needs q columns of block j and
        # k/v columns of blocks 0..j (same batch) -- all computed by the time
        # block j's QKV is done, so one fused loop pipelines everything:
        # xT loads prefetch under attention PE work, and output stores drain
        # under the next block's compute.
        PW = min(512, d)
        NIB = d // PW
        with ExitStack() as body:
            xpool = body.enter_context(tc.tile_pool(name="xpool", bufs=4 if bmode else 3))
            vtpool = body.enter_context(tc.tile_pool(name="vtpool", bufs=2))
            npool = body.enter_context(tc.tile_pool(name="npool", bufs=12 if bmode else 5))
            zpool = body.enter_context(tc.tile_pool(name="zpool", bufs=2))
            tmpool = body.enter_context(tc.tile_pool(name="tmpool", bufs=2))
            opool = body.enter_context(tc.tile_pool(name="opool", bufs=2))
            # PSUM budget (8 banks): qkv 2 + scores 2 + av 2 + tr/proj/bc 2
            ps_qkv = body.enter_context(tc.tile_pool(name="ps_qkv", bufs=2, space="PSUM"))
            ps_s = body.enter_context(tc.tile_pool(name="ps_s", bufs=2, space="PSUM"))
            ps_av = body.enter_context(tc.tile_pool(name="ps_av", bufs=2, space="PSUM"))
            ps_tp = body.enter_context(tc.tile_pool(name="ps_tp", bufs=2, space="PSUM"))

            xt_tiles = {}

            def emit_xt(bb, j, chunked=False):
                col0 = bb * t + j * TBLK
                tsl = slice(col0, col0 + TBLK)
                xt = xpool.tile([128, KT, TBLK], MMDT, tag="xt", name=f"xt_{bb}_{j}")
                if chunked:
                    for kt in range(KT):
                        nc.sync.dma_start(
                            out=xt[:, kt, :],
                            in_=bcst(xT[kt * 128:(kt + 1) * 128, tsl]),
                        )
                else:
                    nc.sync.dma_start(
                        out=xt[:],
                        in_=bcst(xT.rearrange("(kt p) c -> p kt c", p=128)[:, :, tsl]),
                    )
                xt_tiles[(bb, j)] = xt

            def emit_qkv(bb, j):
                col0 = bb * t + j * TBLK
                tsl = slice(col0, col0 + TBLK)
                xt = xt_tiles.pop((bb, j))
                for w_sb, dst in ((wq_sb, qT_sb), (wk_sb, kT_sb)):
                    ps = ps_qkv.tile([128, TBLK], f32, tag="ps_qkv",
                                     name=f"psq_{bb}_{j}_{dst.name}")
                    for kt in range(KT):
                        nc.tensor.matmul(ps[:], w_sb[:, kt, :], xt[:, kt, :],
                                         start=(kt == 0), stop=(kt == KT - 1))
                    nc.vector.tensor_copy(dst[:, tsl], ps[:])
                ps = ps_qkv.tile([128, TBLK], f32, tag="ps_qkv", name=f"psv_{bb}_{j}")
                for kt in range(KT):
                    nc.tensor.matmul(ps[:], wv_sb[:, kt, :], xt[:, kt, :],
                                     start=(kt == 0), stop=(kt == KT - 1))
                vt = vtpool.tile([128, TBLK], MMDT, tag="vt", name=f"vt_{bb}_{j}")
                nc.vector.tensor_copy(vt[:], ps[:])
                for s4 in range(SPT):
                    sb_idx = (col0 // 128) + s4
                    pt = ps_tp.tile([128, 128], MMDT, tag="ps_tp",
                                    name=f"ptr_{bb}_{j}_{s4}")
                    nc.tensor.transpose(pt[:], vt[:, s4 * 128:(s4 + 1) * 128],
                                        ident[:])
                    nc.vector.tensor_copy(
                        vaug[:, sb_idx, :].rearrange(
                            "p (g c) -> p g c", g=HPC)[:, :, 0:HD],
                        pt[:].rearrange("p (g c) -> p g c", g=HPC),
                    )

            def emit_attn(bb, j, last=False):
                col0 = bb * t + j * TBLK
                tsl = slice(col0, col0 + TBLK)
                n_i = (j + 1) * SPT
                avs = [ps_av.tile([65, TBLK], f32, tag="ps_av", name=f"av_{bb}_{j}_{h}")
                       for h in range(HPC)]

                def emit_av(i_, ddp_, nh_):
                    for h in range(HPC):
                        sb_idx = (bb * t + i_ * 128) // 128
                        nc.tensor.matmul(
                            avs[h][:, ddp_:], vaug[:, sb_idx, h * 66:h * 66 + HD + 1],
                            nh_[:, h * TBLK + ddp_:(h + 1) * TBLK],
                            start=(i_ == 0), stop=(i_ == n_i - 1),
                            skip_group_check=True)

                from collections import deque
                pend = deque()
                for i in range(n_i):
                    ssl = slice(bb * t + i * 128, bb * t + i * 128 + 128)
                    dd = 128 * i - TBLK * j
                    # column trim: scores/exp/av touch only cols >= ddp
                    # (f32r needs free dim >= 256 for the PE fast path;
                    # bf16 runs 1 cyc/row at any width so trim fully)
                    if rmode:
                        ddp = max(0, min(dd, TBLK - 256))
                    else:
                        ddp = max(0, dd)
                    nh = npool.tile([128, HPC * TBLK], MMDT, tag="nh",
                                    name=f"nh_{bb}_{j}_{i}")
                    for h in range(HPC):
                        hp = slice(h * HD, (h + 1) * HD)
                        ps = ps_s.tile([128, TBLK], f32, tag="ps_s",
                                       name=f"pss_{bb}_{j}_{i}_{h}")
                        nc.tensor.matmul(
                            ps[:, ddp:], kT_sb[hp, ssl],
                            qT_sb[hp, tsl][:, ddp:], start=True, stop=True)
                        nc.scalar.activation(
                            nh[:, h * TBLK + ddp:(h + 1) * TBLK], ps[:, ddp:],
                            mybir.ActivationFunctionType.Exp, scale=0.125)
                        if dd >= 0:
                            # mask cols [ddp, dd+128): staircase + trim slack
                            # (nh col c maps to mask col c + 384 - dd)
                            nc.gpsimd.tensor_mul(
                                nh[:, h * TBLK + ddp:h * TBLK + dd + 128],
                                nh[:, h * TBLK + ddp:h * TBLK + dd + 128],
                                maskt[:, 384 - dd + ddp:512])
                    # attn@v lags three i-steps so exp (ACT) latency jitter
                    # hides under PE work
                    pend.append((i, ddp, nh))
                    if len(pend) > 5:
                        i_, ddp_, nh_ = pend.popleft()
                        emit_av(i_, ddp_, nh_)
                while pend:
                    i_, ddp_, nh_ = pend.popleft()
                    emit_av(i_, ddp_, nh_)

                # --- softmax normalization: out = av * (1/Z) ---
                rr = zpool.tile([65, HPC * TBLK], MMDT, tag="rr",
                                name=f"rrr_{bb}_{j}")
                with nc.allow_low_precision(reason="f32r PE broadcast of 1/Z"):
                    for h in range(HPC):
                        nc.vector.reciprocal(
                            rr[64:65, h * TBLK:(h + 1) * TBLK], avs[h][64:65, :])
                # h1 first: its outT write goes through a SBUF->SBUF DMA
                # (partition shift), so start it before h0's direct DVE write
                bcs_h = {}
                for h in reversed(range(HPC)):
                    # K=1 matmul broadcasts 1/Z across the 64 output partitions
                    bc = ps_tp.tile([HD, TBLK], f32, tag="ps_tp", name=f"bc_{bb}_{j}_{h}")
                    nc.tensor.matmul(bc[:], one1[64:65, :],
                                     rr[64:65, h * TBLK:(h + 1) * TBLK],
                                     start=True, stop=True)
                    # DVE may read only one PSUM operand: stage bc in SBUF
                    # (on DVE -- ACT is loaded with the exp stream)
                    bcs = tmpool.tile([HD, TBLK], f32, tag="bcs", name=f"bcs_{bb}_{j}_{h}")
                    nc.vector.tensor_copy(bcs[:], bc[:])
                    bcs_h[h] = bcs
                    if h == 0:
                        nc.vector.tensor_mul(outT_sb[0:HD, tsl], avs[h][0:HD, :], bcs[:])
                    else:
                        tmp = tmpool.tile([HD, TBLK], MMDT, tag="tmp", name=f"tm_{bb}_{j}")
                        nc.vector.tensor_mul(tmp[:], avs[h][0:HD, :], bcs[:])
                        nc.sync.dma_start(
                            out=outT_sb[h * HD:(h + 1) * HD, tsl], in_=tmp[:])

            def emit_proj(bb, j, last=False):
                col0 = bb * t + j * TBLK
                NTL = TBLK // 128
                ot = opool.tile([128, NTL, d], OUTDT, tag="ot", name=f"ot_{bb}_{j}")
                for tl in range(NTL):
                    tt = col0 // 128 + tl
                    for ib in range(NIB):
                        ps = ps_tp.tile([128, PW], f32, tag="ps_tp",
                                        name=f"psp_{bb}_{j}_{tl}_{ib}")
                        nc.tensor.matmul(ps[:], outT_sb[:, tt * 128:(tt + 1) * 128],
                                         wp_sb[:, ib * PW:(ib + 1) * PW],
                                         start=True, stop=True)
                        # in the drain there is no exp stream: split copies
                        # between DVE and ACT and store per row-block pair so
                        # the store overlaps the remaining copies
                        if last and (tl * NIB + ib) % 2 == 1:
                            nc.scalar.copy(ot[:, tl, ib * PW:(ib + 1) * PW], ps[:])
                        else:
                            nc.vector.tensor_copy(
                                ot[:, tl, ib * PW:(ib + 1) * PW], ps[:])
                    if last:
                        nc.sync.dma_start(
                            out=out_p.rearrange("(tb p) c -> p tb c", p=128)[
                                :, col0 // 128 + tl:col0 // 128 + tl + 1, :],
                            in_=ot[:, tl:tl + 1, :])
                if not last:
                    # one store DMA per block: [p, tl, d] -> row-blocks of out_p
                    nc.sync.dma_start(
                        out=out_p.rearrange("(tb p) c -> p tb c", p=128)[
                            :, col0 // 128:col0 // 128 + NTL, :],
                        in_=ot[:])

            # software pipeline: QKV runs one t-block ahead of attention, and
            # the projection lags one block behind, so block-boundary DVE/DMA
            # latencies hide under attention PE work
            blocks = [(bb, j) for bb in range(b) for j in range(NJ)]
            emit_xt(*blocks[0], chunked=True)
            emit_xt(*blocks[1])
            emit_qkv(*blocks[0])
            for idx, blk in enumerate(blocks):
                if idx + 2 < len(blocks):
                    emit_xt(*blocks[idx + 2])
                if idx + 1 < len(blocks):
                    emit_qkv(*blocks[idx + 1])
                emit_attn(*blk, last=(idx == len(blocks) - 1))
                if idx >= 1:
                    emit_proj(*blocks[idx - 1], last=(idx == len(blocks) - 1))
            emit_proj(*blocks[-1], last=True)

    nc.compile()
    return nc


def _get_nc(b=B, t=T, d=D, cfg="b"):
    key = (b, t, d, cfg)
    if key not in _CACHE:
        _CACHE[key] = _build(b, t, d, cfg)
    return _CACHE[key]


def _in_dtype(cfg):
    if cfg == "b":
        import ml_dtypes
        return np.dtype(ml_dtypes.bfloat16)
    return np.dtype(np.float32)


def _make_consts(b, t, d, dt):
    bt = b * t
    TBLK = min(512, t)
    NSB = bt // 128
    cident = np.eye(128, dtype=dt)
    p = np.arange(128, dtype=np.int64)[:, None]
    m = np.arange(TBLK + 384, dtype=np.int64)[None, :]
    cmask = (m >= p + 384).astype(dt)
    cones = np.ones((128, NSB, HPC), dtype=dt)
    cone1 = np.ones((65, HD), dtype=dt)
    return {"cident": cident, "cmask": cmask, "cones": cones, "cone1": cone1}


def _prepare_in_maps(x, Wq, Wk, Wv, Wp, b, t, d, cfg):
    bt = b * t
    dt = _in_dtype(cfg)
    xT = np.ascontiguousarray(x.reshape(bt, d).T.astype(dt))
    consts = _make_consts(b, t, d, dt)
    in_maps = []
    for c in range(NCORES):
        h0 = c * HPC
        wq_c = np.ascontiguousarray(Wq[h0:h0 + HPC].reshape(CH, d).T.astype(dt))
        wk_c = np.ascontiguousarray(Wk[h0:h0 + HPC].reshape(CH, d).T.astype(dt))
        wv_c = np.ascontiguousarray(Wv[h0:h0 + HPC].reshape(CH, d).T.astype(dt))
        wp_c = np.ascontiguousarray(Wp[:, c * CH:(c + 1) * CH].T.astype(dt))
        in_maps.append({"xT": xT, "wq": wq_c, "wk": wk_c, "wv": wv_c, "wp": wp_c,
                        **consts})
    return in_maps


def _run(x, Wq, Wk, Wv, Wp, bp, b, t, d, cfg, trace=False):
    from concourse.bass_utils import run_bass_kernel_spmd
    nc = _get_nc(b, t, d, cfg)
    in_maps = _prepare_in_maps(x, Wq, Wk, Wv, Wp, b, t, d, cfg)
    res = run_bass_kernel_spmd(nc, in_maps, core_ids=list(range(NCORES)), trace=trace)
    acc = np.zeros((b * t, d), dtype=np.float64)
    for r in res.results:
        acc += r["out_p"].astype(np.float64)
    out = (acc + np.asarray(bp, dtype=np.float64)).astype(np.float32)
    return out.reshape(b, t, d), res


KERNEL_CFG = "b"


def kernel(x, Wq, Wk, Wv, Wp, bp):
    out, _ = _run(np.asarray(x), np.asarray(Wq), np.asarray(Wk), np.asarray(Wv),
                  np.asarray(Wp), np.asarray(bp), B, T, D, KERNEL_CFG, trace=False)
    return out
